# revision 1
# baseline (speedup 1.0000x reference)
"""Conv2d(128->256, k=3, s=1, VALID) on 8 TRN2 NeuronCores.

Strategy: data-parallel over batch (32 images -> 4 per core). On each core
the conv is 9 shifted matmuls per output tile: contraction over
in_channels=128 sits on the PE partition dim, weights W[:, :, kh, kw]
(transposed to [ic, oc]) are the stationary operand, and the moving operand
is a [128, rows, 110] window of the resident input image in SBUF. PSUM
accumulates the 9 taps (x2 oc halves); bias is fused into the PSUM->SBUF
copy on the scalar engine. Matmuls run as float32r (fp32 bits, full PE
rate for moving dim >= 256).
"""

import numpy as np

import concourse.bass as bass
from concourse import bacc
import concourse.mybir as mybir
import concourse.tile as tile
from concourse.bass_utils import run_bass_kernel_spmd

N_CORES = 8
N, IC, H, W = 32, 128, 112, 112
OC, K = 256, 3
OH, OW = H - K + 1, W - K + 1  # 110, 110
NPC = N // N_CORES  # images per core
OCH = OC // 128  # oc halves

_f32 = mybir.dt.float32
_f32r = mybir.dt.float32r
_bf16 = mybir.dt.bfloat16

# matmul operand dtype: "bf16" (fast path, pipelined ldweights) or "f32r"
MM_DTYPE = "f32r"

# 26 blocks of 4 output rows + 2 blocks of 3 (keeps moving dim >= 256
# so float32r matmuls stay at the 1 cycle/row rate)
ROW_BLOCKS = [(i * 4, 4) for i in range(26)] + [(104, 3), (107, 3)]


def _build_program(npc: int = NPC, reps: int = 1, mm_dtype: str = MM_DTYPE, probe: str | None = None) -> bacc.Bacc:
    _dt = _bf16 if mm_dtype == "bf16" else _f32r
    nc = bacc.Bacc("TRN2", target_bir_lowering=False, debug=False)
    xd = nc.dram_tensor("x", [npc, IC, H * W], _dt, kind="ExternalInput").ap()
    wd = nc.dram_tensor("w", [IC, 9 * OCH * 128], _dt, kind="ExternalInput").ap()
    bd = nc.dram_tensor("b", [128, OCH], _f32, kind="ExternalInput").ap()
    od = nc.dram_tensor("out", [NPC, OC, OH, OW], _f32, kind="ExternalOutput").ap()

    # Input halves: bufA holds rows 0..57, bufB rows 56..111. Every row
    # block's 3x3 taps stay inside one buffer.
    A_ROWS, B_ROWS, B_BASE = 58, 56, 56
    HALF1 = [(i * 4, 4) for i in range(14)]              # oh 0..55
    HALF2 = [(56 + i * 4, 4) for i in range(12)] + [(104, 3), (107, 3)]
    # output quarters -> one out-DMA each (och x quarter)
    QUARTERS = [HALF1[:7], HALF1[7:], HALF2[:7], HALF2[7:]]

    with tile.TileContext(nc) as tc:
        with (
            tc.tile_pool(name="wp", bufs=1) as wp,
            tc.tile_pool(name="xa", bufs=1) as xa_pool,
            tc.tile_pool(name="xb", bufs=1) as xb_pool,
            tc.tile_pool(name="op", bufs=1) as op,
            tc.tile_pool(name="pp", bufs=6, space="PSUM") as pp,
            tc.tile_pool(name="ap", bufs=1, space="PSUM") as absorb_pool,
        ):
            w_sb = wp.tile([128, 9 * OCH * 128], _dt)
            nc.sync.dma_start(w_sb[:], wd[:])
            wv = w_sb[:].rearrange("p (k o c) -> p k o c", k=9, o=OCH)
            b_sb = wp.tile([128, OCH], _f32)
            nc.sync.dma_start(b_sb[:], bd[:])

            # The fused fp32r matmul instruction only has room for ONE
            # semaphore wait, and the activation instruction for TWO. The
            # dummy ops below each absorb one DMA semaphore into the
            # engine's vector clock so the real matmuls/activations never
            # exceed their budget.
            absorb_ps = absorb_pool.tile([128, 8 * 24], _f32)
            absorb_idx = [0]

            def absorb_mm(rhs_ap):
                k = absorb_idx[0] % 24
                absorb_idx[0] += 1
                nc.tensor.matmul(
                    absorb_ps[:, 8 * k : 8 * (k + 1)],
                    lhsT=wv[:, 0, 0, :],
                    rhs=rhs_ap,
                )

            absorb_mm(w_sb[:, :8])

            b_scratch = wp.tile([128, OCH], _f32)
            nc.scalar.activation(
                b_scratch[:], b_sb[:], mybir.ActivationFunctionType.Copy
            )

            # persistent per-image output buffer: each row block writes its
            # own region, so copyback ACTs only wait {PE, self}
            ob = op.tile([128, OCH, OH * OW], _f32)
            obv = ob[:].rearrange("p o (h w) -> p o h w", h=OH)

            xA = xa_pool.tile([128, A_ROWS * W], _dt)
            xB = xb_pool.tile([128, B_ROWS * W], _dt)

            def load_half(buf, n, row0, nrows, nchunks=2):
                rows_per = (nrows + nchunks - 1) // nchunks
                for c in range(nchunks):
                    r0 = c * rows_per
                    r1 = min(nrows, r0 + rows_per)
                    nc.sync.dma_start(
                        buf[:, r0 * W : r1 * W],
                        xd[n, :, (row0 + r0) * W : (row0 + r1) * W],
                    )
                for c in range(nchunks):
                    absorb_mm(buf[:, c * rows_per * W : c * rows_per * W + 8])

            def do_blocks(buf, base_row, blocks, n):
                xv = buf[:].rearrange("p (h w) -> p h w", w=W)
                for oh, rows in blocks:
                    for och in range(OCH):
                        ps = pp.tile([128, 4, OW], _f32)
                        ps_ap = ps[:, :rows, :]
                        for pos in range(9):
                            kh, kw = divmod(pos, 3)
                            r = oh + kh - base_row
                            nc.tensor.matmul(
                                ps_ap,
                                lhsT=wv[:, pos, och, :],
                                rhs=xv[:, r : r + rows, kw : kw + OW],
                                start=(pos == 0),
                                stop=(pos == 8),
                            )
                        if probe != "mmonly":
                            nc.scalar.activation(
                                obv[:, och, oh : oh + rows, :],
                                ps_ap,
                                mybir.ActivationFunctionType.Identity,
                                bias=b_sb[:, och : och + 1],
                            )

            load_half(xA, 0, 0, A_ROWS)
            load_half(xB, 0, B_BASE, B_ROWS)
            for g in range(reps * npc):
                n = g % npc
                if g > 0 and probe is None:
                    # dummy ACTs: absorb the previous image's out-DMA
                    # completion (region recycle) into the ACT clock
                    for och in range(OCH):
                        for q in QUARTERS:
                            oh0 = q[0][0]
                            nc.scalar.activation(
                                obv[:, och, oh0 : oh0 + 1, :1],
                                b_scratch[:, :1],
                                mybir.ActivationFunctionType.Copy,
                            )
                do_blocks(xA, 0, HALF1, n)

                do_blocks(xB, B_BASE, HALF2, n)
                # out-DMAs: one per (och, quarter)
                for och in range(OCH) if probe is None else []:
                    for q in QUARTERS:
                        oh0 = q[0][0]
                        oh1 = q[-1][0] + q[-1][1]
                        nc.sync.dma_start(
                            od[n, och * 128 : (och + 1) * 128, oh0:oh1, :],
                            obv[:, och, oh0:oh1, :],
                        )
                if g + 1 < reps * npc:
                    nxt = (g + 1) % npc
                    load_half(xA, nxt, 0, A_ROWS)
                    load_half(xB, nxt, B_BASE, B_ROWS)
    return nc


def _prep_in_maps(x, weight, bias, mm_dtype: str = MM_DTYPE):
    import ml_dtypes

    dt_np = ml_dtypes.bfloat16 if mm_dtype == "bf16" else np.float32
    x = np.ascontiguousarray(np.asarray(x, dtype=np.float32).astype(dt_np))
    weight = np.asarray(weight, dtype=np.float32)
    bias = np.asarray(bias, dtype=np.float32)

    # [oc, ic, kh, kw] -> [ic, kh*kw, och, oc_in_half] flattened
    wt = np.ascontiguousarray(
        weight.transpose(1, 2, 3, 0).reshape(IC, 9 * OC).astype(dt_np)
    )
    b2 = np.ascontiguousarray(bias.reshape(OCH, 128).T)
    return [
        {
            "x": np.ascontiguousarray(
                x[c * NPC : (c + 1) * NPC].reshape(NPC, IC, H * W)
            ),
            "w": wt,
            "b": b2,
        }
        for c in range(N_CORES)
    ]


def kernel(x: np.ndarray, weight: np.ndarray, bias: np.ndarray) -> np.ndarray:
    nc = _build_program()
    if not nc.is_finalized():
        nc.finalize()
    in_maps = _prep_in_maps(x, weight, bias)
    res = run_bass_kernel_spmd(nc, in_maps, list(range(N_CORES)))
    out = np.concatenate([res.results[c]["out"] for c in range(N_CORES)], axis=0)
    return out



# revision 2
# speedup vs baseline: 1.1391x; 1.1391x over previous
"""Conv2d(128->256, k=3, s=1, VALID) on 8 TRN2 NeuronCores.

Strategy: data-parallel over batch (32 images -> 4 per core). On each core
the conv is 9 shifted matmuls per output tile: contraction over
in_channels=128 sits on the PE partition dim, weights W[:, :, kh, kw]
(transposed to [ic, oc]) are the stationary operand, and the moving operand
is a [128, rows, 110] window of the resident input image in SBUF. PSUM
accumulates the 9 taps (x2 oc halves); bias is fused into the PSUM->SBUF
copy on the scalar engine. Matmuls run as float32r (fp32 bits, full PE
rate for moving dim >= 256).
"""

import numpy as np

import concourse.bass as bass
from concourse import bacc
import concourse.mybir as mybir
import concourse.tile as tile
from concourse.bass_utils import run_bass_kernel_spmd

N_CORES = 8
N, IC, H, W = 32, 128, 112, 112
OC, K = 256, 3
OH, OW = H - K + 1, W - K + 1  # 110, 110
NPC = N // N_CORES  # images per core
OCH = OC // 128  # oc halves

_f32 = mybir.dt.float32
_f32r = mybir.dt.float32r
_bf16 = mybir.dt.bfloat16

# matmul operand dtype: "bf16" (fast path, pipelined ldweights) or "f32r"
MM_DTYPE = "bf16"

# 26 blocks of 4 output rows + 2 blocks of 3 (keeps moving dim >= 256
# so float32r matmuls stay at the 1 cycle/row rate)
ROW_BLOCKS = [(i * 4, 4) for i in range(26)] + [(104, 3), (107, 3)]


def _build_program(npc: int = NPC, reps: int = 1, mm_dtype: str = MM_DTYPE, probe: str | None = None) -> bacc.Bacc:
    _dt = _bf16 if mm_dtype == "bf16" else _f32r
    nc = bacc.Bacc("TRN2", target_bir_lowering=False, debug=False)
    xd = nc.dram_tensor("x", [npc, IC, H * W], _dt, kind="ExternalInput").ap()
    wd = nc.dram_tensor("w", [IC, 9 * OCH * 128], _dt, kind="ExternalInput").ap()
    bd = nc.dram_tensor("b", [128, OCH], _f32, kind="ExternalInput").ap()
    od = nc.dram_tensor("out", [NPC, OC, OH, OW], _f32, kind="ExternalOutput").ap()

    # Input halves: bufA holds rows 0..57, bufB rows 56..111. Every row
    # block's 3x3 taps stay inside one buffer.
    A_ROWS, B_ROWS, B_BASE = 58, 56, 56
    HALF1 = [(i * 4, 4) for i in range(14)]              # oh 0..55
    HALF2 = [(56 + i * 4, 4) for i in range(12)] + [(104, 3), (107, 3)]
    # output quarters -> one out-DMA each (och x quarter)
    QUARTERS = [HALF1[:7], HALF1[7:], HALF2[:7], HALF2[7:]]

    with tile.TileContext(nc) as tc:
        with (
            tc.tile_pool(name="wp", bufs=1) as wp,
            tc.tile_pool(name="xa", bufs=1) as xa_pool,
            tc.tile_pool(name="xb", bufs=1) as xb_pool,
            tc.tile_pool(name="op", bufs=1) as op,
            tc.tile_pool(name="pp", bufs=6, space="PSUM") as pp,
            tc.tile_pool(name="ap", bufs=1, space="PSUM") as absorb_pool,
        ):
            w_sb = wp.tile([128, 9 * OCH * 128], _dt)
            nc.sync.dma_start(w_sb[:], wd[:])
            wv = w_sb[:].rearrange("p (k o c) -> p k o c", k=9, o=OCH)
            b_sb = wp.tile([128, OCH], _f32)
            nc.sync.dma_start(b_sb[:], bd[:])

            # The fused fp32r matmul instruction only has room for ONE
            # semaphore wait, and the activation instruction for TWO. The
            # dummy ops below each absorb one DMA semaphore into the
            # engine's vector clock so the real matmuls/activations never
            # exceed their budget.
            absorb_ps = absorb_pool.tile([128, 8 * 24], _f32)
            absorb_idx = [0]

            def absorb_mm(rhs_ap):
                k = absorb_idx[0] % 24
                absorb_idx[0] += 1
                nc.tensor.matmul(
                    absorb_ps[:, 8 * k : 8 * (k + 1)],
                    lhsT=wv[:, 0, 0, :],
                    rhs=rhs_ap,
                )

            absorb_mm(w_sb[:, :8])

            b_scratch = wp.tile([128, OCH], _f32)
            nc.scalar.activation(
                b_scratch[:], b_sb[:], mybir.ActivationFunctionType.Copy
            )

            # persistent per-image output buffer: each row block writes its
            # own region, so copyback ACTs only wait {PE, self}
            ob = op.tile([128, OCH, OH * OW], _f32)
            obv = ob[:].rearrange("p o (h w) -> p o h w", h=OH)

            xA = xa_pool.tile([128, A_ROWS * W], _dt)
            xB = xb_pool.tile([128, B_ROWS * W], _dt)

            def load_half(buf, n, row0, nrows, nchunks=2):
                rows_per = (nrows + nchunks - 1) // nchunks
                for c in range(nchunks):
                    r0 = c * rows_per
                    r1 = min(nrows, r0 + rows_per)
                    nc.sync.dma_start(
                        buf[:, r0 * W : r1 * W],
                        xd[n, :, (row0 + r0) * W : (row0 + r1) * W],
                    )
                for c in range(nchunks):
                    absorb_mm(buf[:, c * rows_per * W : c * rows_per * W + 8])

            def do_blocks(buf, base_row, blocks, n):
                xv = buf[:].rearrange("p (h w) -> p h w", w=W)
                for oh, rows in blocks:
                    for och in range(OCH):
                        ps = pp.tile([128, 4, OW], _f32)
                        ps_ap = ps[:, :rows, :]
                        for pos in range(9):
                            kh, kw = divmod(pos, 3)
                            r = oh + kh - base_row
                            nc.tensor.matmul(
                                ps_ap,
                                lhsT=wv[:, pos, och, :],
                                rhs=xv[:, r : r + rows, kw : kw + OW],
                                start=(pos == 0),
                                stop=(pos == 8),
                            )
                        if probe != "mmonly":
                            nc.scalar.activation(
                                obv[:, och, oh : oh + rows, :],
                                ps_ap,
                                mybir.ActivationFunctionType.Identity,
                                bias=b_sb[:, och : och + 1],
                            )

            load_half(xA, 0, 0, A_ROWS)
            load_half(xB, 0, B_BASE, B_ROWS)
            for g in range(reps * npc):
                n = g % npc
                if g > 0 and probe is None:
                    # dummy ACTs: absorb the previous image's out-DMA
                    # completion (region recycle) into the ACT clock
                    for och in range(OCH):
                        for q in QUARTERS:
                            oh0 = q[0][0]
                            nc.scalar.activation(
                                obv[:, och, oh0 : oh0 + 1, :1],
                                b_scratch[:, :1],
                                mybir.ActivationFunctionType.Copy,
                            )
                do_blocks(xA, 0, HALF1, n)

                do_blocks(xB, B_BASE, HALF2, n)
                # out-DMAs: one per (och, quarter)
                for och in range(OCH) if probe is None else []:
                    for q in QUARTERS:
                        oh0 = q[0][0]
                        oh1 = q[-1][0] + q[-1][1]
                        nc.sync.dma_start(
                            od[n, och * 128 : (och + 1) * 128, oh0:oh1, :],
                            obv[:, och, oh0:oh1, :],
                        )
                if g + 1 < reps * npc:
                    nxt = (g + 1) % npc
                    load_half(xA, nxt, 0, A_ROWS)
                    load_half(xB, nxt, B_BASE, B_ROWS)
    return nc


def _prep_in_maps(x, weight, bias, mm_dtype: str = MM_DTYPE):
    import ml_dtypes

    dt_np = ml_dtypes.bfloat16 if mm_dtype == "bf16" else np.float32
    x = np.ascontiguousarray(np.asarray(x, dtype=np.float32).astype(dt_np))
    weight = np.asarray(weight, dtype=np.float32)
    bias = np.asarray(bias, dtype=np.float32)

    # [oc, ic, kh, kw] -> [ic, kh*kw, och, oc_in_half] flattened
    wt = np.ascontiguousarray(
        weight.transpose(1, 2, 3, 0).reshape(IC, 9 * OC).astype(dt_np)
    )
    b2 = np.ascontiguousarray(bias.reshape(OCH, 128).T)
    return [
        {
            "x": np.ascontiguousarray(
                x[c * NPC : (c + 1) * NPC].reshape(NPC, IC, H * W)
            ),
            "w": wt,
            "b": b2,
        }
        for c in range(N_CORES)
    ]


def kernel(x: np.ndarray, weight: np.ndarray, bias: np.ndarray) -> np.ndarray:
    nc = _build_program()
    if not nc.is_finalized():
        nc.finalize()
    in_maps = _prep_in_maps(x, weight, bias)
    res = run_bass_kernel_spmd(nc, in_maps, list(range(N_CORES)))
    out = np.concatenate([res.results[c]["out"] for c in range(N_CORES)], axis=0)
    return out



# revision 3
# speedup vs baseline: 1.1682x; 1.0255x over previous
"""Conv2d(128->256, k=3, s=1, VALID) on 8 TRN2 NeuronCores.

Strategy: data-parallel over batch (32 images -> 4 per core). On each core
the conv is 9 shifted matmuls per output tile: contraction over
in_channels=128 sits on the PE partition dim, weights W[:, :, kh, kw]
(transposed to [ic, oc]) are the stationary operand, and the moving operand
is a [128, rows, 110] window of the resident input image in SBUF. PSUM
accumulates the 9 taps (x2 oc halves). Operands are bf16: full PE rate,
fast-weight-load keeps LDWEIGHTS hidden, and (unlike f32r) the chip does
not hit the P0 power throttle. PSUM->SBUF evacuation is split across the
scalar engine (och0, fused bias via activation) and the vector engine
(och1, tensor_scalar_add bias) so the tail drain halves. Head loads go on
both HW-DGE rings (w/bias on the Activation ring, x chunks on the Sync
ring) with absorbs interleaved so the first matmul starts ~2us in.
"""

import numpy as np

import concourse.bass as bass
from concourse import bacc
import concourse.mybir as mybir
import concourse.tile as tile
from concourse.bass_utils import run_bass_kernel_spmd

N_CORES = 8
N, IC, H, W = 32, 128, 112, 112
OC, K = 256, 3
OH, OW = H - K + 1, W - K + 1  # 110, 110
NPC = N // N_CORES  # images per core
OCH = OC // 128  # oc halves

_f32 = mybir.dt.float32
_f32r = mybir.dt.float32r
_bf16 = mybir.dt.bfloat16

# matmul operand dtype: "bf16" (fast path, pipelined ldweights) or "f32r"
MM_DTYPE = "bf16"

# 26 blocks of 4 output rows + 2 blocks of 3 (keeps moving dim >= 256
# so matmuls stay at the 1 cycle/row rate)
ROW_BLOCKS = [(i * 4, 4) for i in range(26)] + [(104, 3), (107, 3)]


def _build_program(npc: int = NPC, reps: int = 1, mm_dtype: str = MM_DTYPE, probe: str | None = None) -> bacc.Bacc:
    _dt = _bf16 if mm_dtype == "bf16" else _f32r
    nc = bacc.Bacc("TRN2", target_bir_lowering=False, debug=False)
    xd = nc.dram_tensor("x", [npc, IC, H * W], _dt, kind="ExternalInput").ap()
    wd = nc.dram_tensor("w", [IC, 9 * OCH * 128], _dt, kind="ExternalInput").ap()
    bd = nc.dram_tensor("b", [128, OCH], _f32, kind="ExternalInput").ap()
    od = nc.dram_tensor("out", [NPC, OC, OH, OW], _f32, kind="ExternalOutput").ap()

    # Input halves: bufA holds rows 0..57, bufB rows 56..111. Every row
    # block's 3x3 taps stay inside one buffer.
    A_ROWS, B_ROWS, B_BASE = 58, 56, 56
    HALF1 = [(i * 4, 4) for i in range(14)]              # oh 0..55
    HALF2 = [(56 + i * 4, 4) for i in range(12)] + [(104, 3), (107, 3)]
    # output quarters -> one out-DMA each (och x quarter)
    QUARTERS = [HALF1[:7], HALF1[7:], HALF2[:7], HALF2[7:]]

    with tile.TileContext(nc) as tc:
        with (
            tc.tile_pool(name="wp", bufs=1) as wp,
            tc.tile_pool(name="xa", bufs=1) as xa_pool,
            tc.tile_pool(name="xb", bufs=1) as xb_pool,
            tc.tile_pool(name="op", bufs=1) as op,
            tc.tile_pool(name="pp", bufs=6, space="PSUM") as pp,
            tc.tile_pool(name="ap", bufs=1, space="PSUM") as absorb_pool,
        ):
            # w + bias on the Activation HW-DGE ring so they overlap the
            # first x chunks on the Sync ring.
            w_sb = wp.tile([128, 9 * OCH * 128], _dt)
            nc.scalar.dma_start(w_sb[:], wd[:])
            wv = w_sb[:].rearrange("p (k o c) -> p k o c", k=9, o=OCH)
            b_sb = wp.tile([128, OCH], _f32)
            nc.scalar.dma_start(b_sb[:], bd[:])

            # Engine wait-slot budget: the matmul instruction only has room
            # for ONE semaphore wait, activation / vector ops for TWO. The
            # dummy ops below each absorb one DMA semaphore into the
            # engine's vector clock so the real matmuls/activations never
            # exceed their budget.
            absorb_ps = absorb_pool.tile([128, 8 * 24], _f32)
            absorb_idx = [0]

            def absorb_mm(rhs_ap):
                k = absorb_idx[0] % 24
                absorb_idx[0] += 1
                nc.tensor.matmul(
                    absorb_ps[:, 8 * k : 8 * (k + 1)],
                    lhsT=wv[:, 0, 0, :],
                    rhs=rhs_ap,
                )

            absorb_mm(w_sb[:, :8])

            b_scratch = wp.tile([128, OCH], _f32)
            nc.scalar.activation(
                b_scratch[:], b_sb[:], mybir.ActivationFunctionType.Copy
            )

            # persistent per-image output buffer: each row block writes its
            # own region, so copyback ops only wait {PE, self}
            ob = op.tile([128, OCH, OH * OW], _f32)
            obv = ob[:].rearrange("p o (h w) -> p o h w", h=OH)

            xA = xa_pool.tile([128, A_ROWS * W], _dt)
            xB = xb_pool.tile([128, B_ROWS * W], _dt)

            def load_half(buf, n, row0, nrows, nchunks=2):
                rows_per = (nrows + nchunks - 1) // nchunks
                for c in range(nchunks):
                    r0 = c * rows_per
                    r1 = min(nrows, r0 + rows_per)
                    nc.sync.dma_start(
                        buf[:, r0 * W : r1 * W],
                        xd[n, :, (row0 + r0) * W : (row0 + r1) * W],
                    )
                for c in range(nchunks):
                    absorb_mm(buf[:, c * rows_per * W : c * rows_per * W + 8])

            def evac(och, oh, rows, ps_ap):
                # och0 -> scalar (fused bias activation), och1 -> vector
                # (tensor_scalar add). Two engines drain PSUM in parallel.
                if och == 0:
                    nc.scalar.activation(
                        obv[:, och, oh : oh + rows, :],
                        ps_ap,
                        mybir.ActivationFunctionType.Identity,
                        bias=b_sb[:, och : och + 1],
                    )
                else:
                    nc.vector.tensor_scalar_add(
                        obv[:, och, oh : oh + rows, :],
                        ps_ap,
                        b_sb[:, och : och + 1],
                    )

            def do_blocks(buf, base_row, blocks, n):
                xv = buf[:].rearrange("p (h w) -> p h w", w=W)
                for oh, rows in blocks:
                    for och in range(OCH):
                        ps = pp.tile([128, 4, OW], _f32)
                        ps_ap = ps[:, :rows, :]
                        for pos in range(9):
                            kh, kw = divmod(pos, 3)
                            r = oh + kh - base_row
                            nc.tensor.matmul(
                                ps_ap,
                                lhsT=wv[:, pos, och, :],
                                rhs=xv[:, r : r + rows, kw : kw + OW],
                                start=(pos == 0),
                                stop=(pos == 8),
                            )
                        if probe != "mmonly":
                            evac(och, oh, rows, ps_ap)

            # First image, first half: 4 chunks with absorbs interleaved so
            # the PE starts on chunk 0 (~15 rows, ~1.2us) instead of the
            # whole 58-row half.
            NCH0 = 4
            rows_per0 = (A_ROWS + NCH0 - 1) // NCH0  # 15
            for c in range(NCH0):
                r0 = c * rows_per0
                r1 = min(A_ROWS, r0 + rows_per0)
                nc.sync.dma_start(
                    xA[:, r0 * W : r1 * W], xd[0, :, r0 * W : r1 * W]
                )
            load_half(xB, 0, B_BASE, B_ROWS)

            def h1_chunked(n):
                # process HALF1 blocks as soon as their chunk has landed
                done = 0
                for c in range(NCH0):
                    top = min(A_ROWS, (c + 1) * rows_per0)
                    absorb_mm(xA[:, c * rows_per0 * W : c * rows_per0 * W + 8])
                    ready = [b for b in HALF1 if b[0] + b[1] + K - 1 <= top]
                    do_blocks(xA, 0, ready[done:], n)
                    done = len(ready)

            for g in range(reps * npc):
                n = g % npc
                last = g + 1 == reps * npc
                if g > 0 and probe is None:
                    # dummy ops: absorb the previous image's out-DMA
                    # completion (region recycle) into each evacuating
                    # engine's clock (scalar for och0, vector for och1)
                    for q in QUARTERS:
                        oh0 = q[0][0]
                        nc.scalar.activation(
                            obv[:, 0, oh0 : oh0 + 1, :1],
                            b_scratch[:, :1],
                            mybir.ActivationFunctionType.Copy,
                        )
                        nc.vector.tensor_scalar_add(
                            obv[:, 1, oh0 : oh0 + 1, :1],
                            b_scratch[:, :1],
                            0.0,
                        )
                if g == 0:
                    h1_chunked(n)
                else:
                    do_blocks(xA, 0, HALF1, n)

                do_blocks(xB, B_BASE, HALF2, n)
                # out-DMAs: one per (quarter, och), interleaved in quarter
                # completion order. Last image: 2-block pieces so the final
                # transfer after the last evacuation is small.
                for q in QUARTERS if probe is None else []:
                    pieces = [q[j : j + 2] for j in range(0, len(q), 2)] if last else [q]
                    for och in range(OCH):
                        for piece in pieces:
                            oh0 = piece[0][0]
                            oh1 = piece[-1][0] + piece[-1][1]
                            nc.sync.dma_start(
                                od[n, och * 128 : (och + 1) * 128, oh0:oh1, :],
                                obv[:, och, oh0:oh1, :],
                            )
                if not last:
                    nxt = (g + 1) % npc
                    load_half(xA, nxt, 0, A_ROWS)
                    load_half(xB, nxt, B_BASE, B_ROWS)
    return nc


def _prep_in_maps(x, weight, bias, mm_dtype: str = MM_DTYPE):
    import ml_dtypes

    dt_np = ml_dtypes.bfloat16 if mm_dtype == "bf16" else np.float32
    x = np.ascontiguousarray(np.asarray(x, dtype=np.float32).astype(dt_np))
    weight = np.asarray(weight, dtype=np.float32)
    bias = np.asarray(bias, dtype=np.float32)

    # [oc, ic, kh, kw] -> [ic, kh*kw, och, oc_in_half] flattened
    wt = np.ascontiguousarray(
        weight.transpose(1, 2, 3, 0).reshape(IC, 9 * OC).astype(dt_np)
    )
    b2 = np.ascontiguousarray(bias.reshape(OCH, 128).T)
    return [
        {
            "x": np.ascontiguousarray(
                x[c * NPC : (c + 1) * NPC].reshape(NPC, IC, H * W)
            ),
            "w": wt,
            "b": b2,
        }
        for c in range(N_CORES)
    ]


def kernel(x: np.ndarray, weight: np.ndarray, bias: np.ndarray) -> np.ndarray:
    nc = _build_program()
    if not nc.is_finalized():
        nc.finalize()
    in_maps = _prep_in_maps(x, weight, bias)
    res = run_bass_kernel_spmd(nc, in_maps, list(range(N_CORES)))
    out = np.concatenate([res.results[c]["out"] for c in range(N_CORES)], axis=0)
    return out
